# revision 14
# baseline (speedup 1.0000x reference)
"""Trainium2 Bass kernel for nn_MultiHeadAttention (B=4, S=2048, D=512, H=8).

Sharding: tensor-parallel over heads — core c owns head c (Dh=64). Each core
computes q/k/v projections for its head slice (full x replicated, host-pre-
transposed to x^T in bf16), attention for its head over all 4 batches, and
the unnormalized partial out-projection O_c @ Wo[c]; the host divides each
core's partial by its softmax denominators (shipped alongside as a [B,S]
vector), sums the 8 partials, and adds the biases that commute with that
reduction (bo, bv@Wo). All on-core compute is bf16 (fp8 blows the 2e-2
error budget: each fp8-quantized operand alone contributes ~2.5%).

Engine plan (emission order IS the per-engine execution order):
  - PE: projections (W-stationary bf16, batch pairs packed on array column
    halves), row-quadrant-alternating S^T (tile_position (hb*64, 0)) so
    weight loads overlap execution, AV with the ones column of V_aug
    producing softmax denominators in PSUM row 64, out-projection.
  - ACT: exclusively exp(S/8) on [128,1024] tiles (its floor, ~140us).
  - DVE: all PSUM evacuations (no reciprocal/normalize on-core).
  - V reaches its [key, dh] AV layout via DMA XBAR transposes into full
    [128, 80] tiles (strided-slot XBAR destinations are broken on HW);
    row 64 of the padded V^T staging tile carries the ones column.
  - Normalization happens on host; output partial + denominators are bf16.
Batches are paired on SBUF partition halves; pair-1 prep fills PE slack
during pair-0 attention, out-projections fill during pair-1 attention.
"""
import numpy as np

import concourse.bass as bass
import concourse.mybir as mybir
import concourse.tile as tile
from concourse import bacc
from concourse.bass_utils import run_bass_kernel_spmd

B, S, D = 4, 2048, 512
H, DH = 8, 64
NCORES = 8
F32 = mybir.dt.float32
BF16 = mybir.dt.bfloat16
AF = mybir.ActivationFunctionType

NKT = S // 128          # 16 key tiles per batch
NQB = S // 512          # 4 query blocks per batch
NCH = D // 128          # 4 dm chunks

_NC_CACHE = {}


def build_kernel():
    nc = bacc.Bacc("TRN2", target_bir_lowering=False, debug=False)

    xT = nc.dram_tensor("xT", [B, D, S], BF16, kind="ExternalInput")
    wq = nc.dram_tensor("wq", [D, DH], BF16, kind="ExternalInput")
    wk = nc.dram_tensor("wk", [D, DH], BF16, kind="ExternalInput")
    wv = nc.dram_tensor("wv", [D, DH], BF16, kind="ExternalInput")
    wo = nc.dram_tensor("wo", [DH, D], BF16, kind="ExternalInput")
    bq = nc.dram_tensor("bq", [128, 1], F32, kind="ExternalInput")
    bk = nc.dram_tensor("bk", [128, 1], F32, kind="ExternalInput")
    onesrow = nc.dram_tensor("onesrow", [1, 2048], BF16, kind="ExternalInput")
    out = nc.dram_tensor("out", [B * S, D], BF16, kind="ExternalOutput")
    dnm = nc.dram_tensor("dnm", [B, S], BF16, kind="ExternalOutput")

    with tile.TileContext(nc) as tc:
        with (
            tc.tile_pool(name="consts", bufs=1) as consts,
            tc.tile_pool(name="xtp", bufs=16) as xtp,
            tc.tile_pool(name="qkp", bufs=2) as qkp,
            tc.tile_pool(name="vtp", bufs=2) as vtp,
            tc.tile_pool(name="vp", bufs=34) as vp,
            tc.tile_pool(name="ptp", bufs=3) as ptp,
            tc.tile_pool(name="otp", bufs=3) as otp,
            tc.tile_pool(name="sop", bufs=4) as sopp,
            tc.tile_pool(name="psA", bufs=2, space="PSUM") as psA,   # pst [128,1024] f32
            tc.tile_pool(name="psO", bufs=2, space="PSUM") as psO,   # po [65,512] f32
            tc.tile_pool(name="psM", bufs=2, space="PSUM") as psM,   # misc [128,512] f32
        ):
            wq_sb = consts.tile([128, NCH, DH], BF16)
            wk_sb = consts.tile([128, NCH, DH], BF16)
            wv_sb = consts.tile([128, NCH, DH], BF16)
            wo_sb = consts.tile([DH, D], BF16)
            bq_sb = consts.tile([128, 1], F32)
            bk_sb = consts.tile([128, 1], F32)
            warm = consts.tile([128, 1], BF16)
            nc.sync.dma_start(out=wq_sb[:], in_=wq.rearrange("(c p) m -> p c m", p=128))
            nc.sync.dma_start(out=wk_sb[:], in_=wk.rearrange("(c p) m -> p c m", p=128))
            nc.sync.dma_start(out=wv_sb[:], in_=wv.rearrange("(c p) m -> p c m", p=128))
            nc.sync.dma_start(out=wo_sb[:], in_=wo[:])
            nc.sync.dma_start(out=bq_sb[:], in_=bq[:])
            nc.sync.dma_start(out=bk_sb[:], in_=bk[:])
            # warmup: pulls the Exp table load (~1.3us) into the kernel head
            nc.scalar.activation(warm[:], bq_sb[:], AF.Exp, scale=0.125)

            state = {}

            def alloc_pair(pr):
                st = {"xt": {}, "vt": {}, "v": {}, "ot": {}}
                st["qt"] = qkp.tile([128, S], BF16, tag="qt", name=f"qt_{pr}")
                st["kt"] = qkp.tile([128, S], BF16, tag="kt", name=f"kt_{pr}")
                state[pr] = st

            def emit_xt_loads(pr):
                st = state[pr]
                for half in range(2):
                    b = pr * 2 + half
                    xts = []
                    for ci in range(NCH):
                        xt_c = xtp.tile([128, S], BF16, tag="xt", name=f"xt_{b}_{ci}")
                        xts.append(xt_c)
                    for blk in range(NQB):
                        for ci in range(NCH):
                            eng = nc.sync if (ci % 2 == 0) else nc.gpsimd
                            eng.dma_start(
                                out=xts[ci][:, bass.ts(blk, 512)],
                                in_=xT[b, bass.ts(ci, 128), bass.ts(blk, 512)],
                            )
                    st["xt"][half] = xts

            def emit_prep_q(pr, blk):
                st = state[pr]
                sl = bass.ts(blk, 512)
                pq = psM.tile([128, 512], F32, tag="psM", name=f"pq_{pr}_{blk}")
                for ci in range(NCH):
                    for half in range(2):
                        nc.tensor.matmul(
                            pq[half * DH:(half + 1) * DH, :],
                            wq_sb[:, ci, :], st["xt"][half][ci][:, sl],
                            start=(ci == 0), stop=(ci == NCH - 1),
                            tile_position=(0, half * DH),
                        )
                nc.vector.tensor_scalar_add(st["qt"][:, sl], pq[:], bq_sb[:])

            def emit_prep_k(pr, blk):
                st = state[pr]
                sl = bass.ts(blk, 512)
                pk = psM.tile([128, 512], F32, tag="psM", name=f"pk_{pr}_{blk}")
                for ci in range(NCH):
                    for half in range(2):
                        nc.tensor.matmul(
                            pk[half * DH:(half + 1) * DH, :],
                            wk_sb[:, ci, :], st["xt"][half][ci][:, sl],
                            start=(ci == 0), stop=(ci == NCH - 1),
                            tile_position=(0, half * DH),
                        )
                nc.vector.tensor_scalar_add(st["kt"][:, sl], pk[:], bk_sb[:])

            def emit_prep_v(pr, blk):
                st = state[pr]
                sl = bass.ts(blk, 512)
                if blk == 0:
                    for half in range(2):
                        b = pr * 2 + half
                        vt_b = vtp.tile([80, S], BF16, tag="vt", name=f"vt_{b}")
                        nc.gpsimd.dma_start(out=vt_b[DH:DH + 1, :], in_=onesrow[:])
                        st["vt"][half] = vt_b
                        st["v"][half] = [None] * NKT
                pv = psM.tile([128, 512], F32, tag="psM", name=f"pv_{pr}_{blk}")
                for ci in range(NCH):
                    for half in range(2):
                        nc.tensor.matmul(
                            pv[half * DH:(half + 1) * DH, :],
                            wv_sb[:, ci, :], st["xt"][half][ci][:, sl],
                            start=(ci == 0), stop=(ci == NCH - 1),
                            tile_position=(0, half * DH),
                        )
                nc.vector.tensor_copy(st["vt"][0][0:DH, sl], pv[0:DH, :])
                nc.vector.tensor_copy(st["vt"][1][0:DH, sl], pv[DH:128, :])
                for half in range(2):
                    b = pr * 2 + half
                    for t in range(blk * 4, blk * 4 + 4):
                        v_t = vp.tile([128, 80], BF16, tag="v", name=f"v_{b}_{t}")
                        nc.sync.dma_start_transpose(
                            out=v_t[:], in_=st["vt"][half][:, bass.ts(t, 128)]
                        )
                        st["v"][half][t] = v_t

            def emit_attn_qq(pr, qq, fillers=None, n_fill=1):
                st = state[pr]
                with nc.named_scope(f"attn_{pr}_{qq}"):
                    sl_q = bass.ts(qq, 512)
                    if qq == 0:
                        for half in range(2):
                            st["ot"][half] = otp.tile(
                                [DH + 1, S], BF16, tag="ot", name=f"ot_{pr * 2 + half}"
                            )
                    po = [
                        psO.tile([DH + 1, 512], F32, tag="psO", name=f"po{hb}_{pr}_{qq}")
                        for hb in range(2)
                    ]
                    for kt_i in range(NKT):
                        kt_sl = bass.ts(kt_i, 128)
                        pst = psA.tile([128, 1024], F32, tag="psA", name=f"pst_{pr}_{qq}_{kt_i}")
                        for hb in range(2):
                            nc.tensor.matmul(
                                pst[:, bass.ts(hb, 512)],
                                st["kt"][hb * DH:(hb + 1) * DH, kt_sl],
                                st["qt"][hb * DH:(hb + 1) * DH, sl_q],
                                start=True, stop=True,
                                tile_position=(hb * DH, 0),
                            )
                        ptt = ptp.tile([128, 1024], BF16, tag="pt", name=f"ptt_{pr}_{qq}_{kt_i}")
                        nc.scalar.activation(ptt[:], pst[:], AF.Exp, scale=0.125)
                        for hb in range(2):
                            nc.tensor.matmul(
                                po[hb][:],
                                st["v"][hb][kt_i][:, 0:DH + 1],
                                ptt[:, bass.ts(hb, 512)],
                                start=(kt_i == 0), stop=(kt_i == NKT - 1),
                            )
                        if fillers is not None:
                            for _ in range(n_fill):
                                if fillers:
                                    fillers.pop(0)()
                    for hb in range(2):
                        nc.vector.tensor_copy(st["ot"][hb][:, sl_q], po[hb][:])

            def emit_op_tt(pr, half, tt):
                st = state[pr]
                b = pr * 2 + half
                ot_b = st["ot"][half]
                pop = psM.tile([128, 512], F32, tag="psM", name=f"pop_{b}_{tt}")
                nc.tensor.matmul(
                    pop[:], ot_b[0:DH, bass.ts(tt, 128)], wo_sb[:],
                    start=True, stop=True,
                )
                so = sopp.tile([128, 512], BF16, tag="so", name=f"so_{b}_{tt}")
                nc.vector.tensor_copy(so[:], pop[:])
                nc.gpsimd.dma_start(
                    out=out[bass.ds(b * S + tt * 128, 128), :], in_=so[:]
                )

            def emit_dnm_dma(pr, half):
                b = pr * 2 + half
                nc.gpsimd.dma_start(
                    out=dnm[b:b + 1, :], in_=state[pr]["ot"][half][DH:DH + 1, :]
                )

            # ---------------- emission schedule ----------------
            import functools
            P = functools.partial
            alloc_pair(0)
            alloc_pair(1)
            emit_xt_loads(0)

            # minimal pair-0 head: block 0 of q/k/v (+ first 8 V transposes)
            emit_prep_q(0, 0)
            emit_prep_k(0, 0)
            emit_prep_v(0, 0)
            emit_xt_loads(1)

            fill0 = []
            for blk in (1, 2, 3):
                fill0.append(P(emit_prep_k, 0, blk))
                fill0.append(P(emit_prep_v, 0, blk))
                fill0.append(P(emit_prep_q, 0, blk))
            for blk in range(NQB):
                fill0.append(P(emit_prep_k, 1, blk))
                fill0.append(P(emit_prep_v, 1, blk))
                fill0.append(P(emit_prep_q, 1, blk))

            emit_attn_qq(0, 0, fillers=fill0, n_fill=1)
            for qq in range(1, NQB):
                emit_attn_qq(0, qq, fillers=fill0, n_fill=1)
            fill1 = fill0   # leftover pair-1 prep, then out-projections
            for half in range(2):
                for tt in range(NKT):
                    fill1.append(P(emit_op_tt, 0, half, tt))
                fill1.append(P(emit_dnm_dma, 0, half))
            for qq in range(NQB):
                emit_attn_qq(1, qq, fillers=fill1, n_fill=1)
                if qq >= 1:
                    for half in range(2):
                        for tt in range((qq - 1) * 4, qq * 4):
                            fill1.append(P(emit_op_tt, 1, half, tt))
            while fill1:
                fill1.pop(0)()
            for half in range(2):
                for tt in range(12, 16):
                    emit_op_tt(1, half, tt)
                emit_dnm_dma(1, half)

    nc.compile()
    return nc


def kernel(x, Wq, bq, Wk, bk, Wv, bv, Wo, bo):
    import ml_dtypes
    BF = ml_dtypes.bfloat16
    x = np.asarray(x, dtype=np.float32)
    xT = np.ascontiguousarray(np.transpose(x, (0, 2, 1))).astype(BF)
    Wq = np.asarray(Wq, dtype=np.float32)
    Wk = np.asarray(Wk, dtype=np.float32)
    Wv = np.asarray(Wv, dtype=np.float32)
    Wo = np.asarray(Wo, dtype=np.float32)
    bq = np.asarray(bq, dtype=np.float32)
    bk = np.asarray(bk, dtype=np.float32)
    bv = np.asarray(bv, dtype=np.float32)
    bo = np.asarray(bo, dtype=np.float32)

    if "nc" not in _NC_CACHE:
        _NC_CACHE["nc"] = build_kernel()
    nc = _NC_CACHE["nc"]

    ones = np.ones((1, 2048), dtype=BF)
    in_maps = []
    for c in range(NCORES):
        hs = slice(c * DH, (c + 1) * DH)
        in_maps.append({
            "xT": xT,
            "wq": np.ascontiguousarray(Wq[:, hs]).astype(BF),
            "wk": np.ascontiguousarray(Wk[:, hs]).astype(BF),
            "wv": np.ascontiguousarray(Wv[:, hs]).astype(BF),
            "wo": np.ascontiguousarray(Wo[hs, :]).astype(BF),
            "bq": np.ascontiguousarray(np.concatenate([bq[hs], bq[hs]]).reshape(128, 1)),
            "bk": np.ascontiguousarray(np.concatenate([bk[hs], bk[hs]]).reshape(128, 1)),
            "onesrow": ones,
        })

    res = run_bass_kernel_spmd(nc, in_maps, list(range(NCORES)))

    acc = np.zeros((B * S, D), dtype=np.float32)
    for c in range(NCORES):
        o = np.asarray(res.results[c]["out"]).astype(np.float32)
        d = np.asarray(res.results[c]["dnm"]).astype(np.float32)
        acc += o / d.reshape(B * S, 1)
    # biases that commute with the head-reduction, applied at gather time
    acc += bo[None, :] + (bv @ Wo)[None, :]
    return acc.reshape(B, S, D)
